# revision 1
# baseline (speedup 1.0000x reference)
"""GCN edge-prediction kernel for 8 trn2 NeuronCores (Bass/Tile).

Math (per GCNConv layer, PyG semantics with self-loops + symmetric norm):
    h = x @ W;  htil = dinv * h  (row scale)
    out[d] = dinv[d] * sum_{e: s->d, incl self} htil[s] + b
Implemented as:
  - node shard of 6250 rows per core; per-layer bf16 node table AllGathered
  - per dst-block (128 nodes) edge chunks of 128; message rows fetched by
    indirect DMA from the table; scatter-sum via PE matmul with a one-hot
    indicator carrying dinv[dst]
  - decode: gather z rows for both label endpoints, fused mul+reduce on DVE
"""
import os
import sys

sys.path.insert(0, "/opt/trn_rl_repo")

import numpy as np
import ml_dtypes

import concourse.bass as bass
import concourse.bacc as bacc
import concourse.mybir as mybir
import concourse.tile as tile
from concourse.bass_utils import run_bass_kernel_spmd

NC = 8
P = 128
SINGLE_PACKET = bool(int(os.environ.get('GCN_SP', '1')))


def _build_plan(n_nodes, edge_index, edge_label_index, dinv):
    """Host-side graph partitioning: per-core, per-dst-block edge chunks.

    Returns dict with per-core offset arrays, indicator blobs and the uniform
    chunk->block map (same for every core, padded to per-block maxima)."""
    sh = n_nodes // NC          # nodes per core
    nb = (sh + P - 1) // P      # dst blocks per core
    src, dst = edge_index[0].astype(np.int64), edge_index[1].astype(np.int64)
    # self loops handled separately (local diag matmul); not in the edge list

    core = dst // sh
    blk = (dst % sh) // P       # dst block within core
    dl = (dst % sh) % P         # dst lane within block

    # group edges per (core, block)
    counts = np.zeros((NC, nb), np.int64)
    np.add.at(counts, (core, blk), 1)
    kb = np.maximum(1, (counts.max(axis=0) + P - 1) // P)  # chunks per block (uniform)
    nch = int(kb.sum())
    chunk_blk = np.repeat(np.arange(nb), kb)               # chunk -> block map

    offs = np.zeros((NC, P, nch), np.int32)
    ind = np.zeros((NC, P, nch * P), np.float32)
    order = np.lexsort((dl, blk, core))
    src_s, blk_s, dl_s, dst_s = src[order], blk[order], dl[order], dst[order]
    core_s = core[order]
    # chunk start offsets per block
    chunk_start = np.zeros(nb + 1, np.int64)
    chunk_start[1:] = np.cumsum(kb)
    bounds = np.searchsorted(core_s * nb + blk_s, np.arange(NC * nb + 1) * 1.0 - 0.5)
    for c in range(NC):
        for b in range(nb):
            lo, hi = bounds[c * nb + b], bounds[c * nb + b + 1]
            cnt = hi - lo
            if cnt == 0:
                continue
            slot = np.arange(cnt)
            ch = chunk_start[b] + slot // P
            lane = slot % P
            offs[c, lane, ch] = src_s[lo:hi]
            ind[c, lane, ch * P + dl_s[lo:hi]] = dinv[dst_s[lo:hi]]
    ind = ind.astype(ml_dtypes.bfloat16)

    # decode plan
    eln = edge_label_index.shape[1]
    lsh = eln // NC             # labels per core
    ldch = (lsh + P - 1) // P   # label chunks per core
    lpad = ldch * P
    offsA = np.zeros((NC, P, ldch), np.int32)
    offsB = np.zeros((NC, P, ldch), np.int32)
    for c in range(NC):
        a = edge_label_index[0, c * lsh:(c + 1) * lsh].astype(np.int32)
        b_ = edge_label_index[1, c * lsh:(c + 1) * lsh].astype(np.int32)
        a = np.pad(a, (0, lpad - lsh))
        b_ = np.pad(b_, (0, lpad - lsh))
        # label l = ch*P + p  ->  slot (p, ch)
        offsA[c] = a.reshape(ldch, P).T
        offsB[c] = b_.reshape(ldch, P).T
    return dict(sh=sh, nb=nb, kb=kb, nch=nch, chunk_blk=chunk_blk,
                offs=offs, ind=ind, offsA=offsA, offsB=offsB,
                lsh=lsh, ldch=ldch)


def _build_bass(n_nodes, f_in, meta):
    sh, nb, kb, nch, ldch = meta["sh"], meta["nb"], meta["kb"], meta["nch"], meta["ldch"]
    chunk_blk = meta["chunk_blk"]
    f32, bf16, i32 = mybir.dt.float32, mybir.dt.bfloat16, mybir.dt.int32
    KIN = f_in // P             # 256/128 = 2 input chunks

    nc = bacc.Bacc(None, target_bir_lowering=False, debug=False, num_devices=NC)

    xT = nc.dram_tensor("xT", [KIN, P, sh], f32, kind="ExternalInput")
    W0 = nc.dram_tensor("W0", [KIN, P, P], f32, kind="ExternalInput")
    W1 = nc.dram_tensor("W1", [P, P], f32, kind="ExternalInput")
    W2 = nc.dram_tensor("W2", [P, P], f32, kind="ExternalInput")
    bcols = nc.dram_tensor("bcols", [P, 3], f32, kind="ExternalInput")
    dinv_blk = nc.dram_tensor("dinv_blk", [P, nb], f32, kind="ExternalInput")
    ident_in = nc.dram_tensor("ident", [P, P], bf16, kind="ExternalInput")
    diag_in = nc.dram_tensor("diag", [P, nb * P], bf16, kind="ExternalInput")
    ind_in = nc.dram_tensor("ind", [P, nch * P], bf16, kind="ExternalInput")
    offs_in = nc.dram_tensor("offs", [P, nch], i32, kind="ExternalInput")
    offsA_in = nc.dram_tensor("offsA", [P, ldch], i32, kind="ExternalInput")
    offsB_in = nc.dram_tensor("offsB", [P, ldch], i32, kind="ExternalInput")
    logits_out = nc.dram_tensor("logits", [P, ldch], f32, kind="ExternalOutput")

    # internal DRAM
    shard = [nc.dram_tensor(f"shard{l}", [sh, P], bf16) for l in range(4)]
    full = [nc.dram_tensor(f"full{l}", [NC * sh, P], bf16, addr_space="Shared")
            for l in range(4)]

    rg = [list(range(NC))]

    with tile.TileContext(nc) as tc:
        with (
            tc.tile_pool(name="const", bufs=1) as cp,
            tc.tile_pool(name="msg", bufs=12) as mp,
            tc.tile_pool(name="work", bufs=3) as wp,
            tc.tile_pool(name="pagg", bufs=4, space="PSUM") as pagg,
            tc.tile_pool(name="pwm", bufs=2, space="PSUM") as pwm,
            tc.tile_pool(name="ptr", bufs=2, space="PSUM") as ptr,
        ):
            w0 = cp.tile([P, KIN, P], f32)
            for k in range(KIN):
                nc.sync.dma_start(w0[:, k, :], W0[k, :, :])
            w1 = cp.tile([P, P], f32)
            nc.sync.dma_start(w1[:], W1[:])
            w2 = cp.tile([P, P], f32)
            nc.sync.dma_start(w2[:], W2[:])
            bc = cp.tile([P, 3], f32)
            nc.sync.dma_start(bc[:], bcols[:])
            dv = cp.tile([P, nb], f32)
            nc.sync.dma_start(dv[:], dinv_blk[:])
            ident = cp.tile([P, P], bf16)
            nc.sync.dma_start(ident[:], ident_in[:])
            diag = cp.tile([P, nb * P], bf16)
            nc.sync.dma_start(diag[:], diag_in[:])
            shard_sb = cp.tile([P, nb, P], bf16)
            nc.gpsimd.memset(shard_sb[:, nb - 1, :], 0.0)
            ind = cp.tile([P, nch * P], bf16)
            nc.sync.dma_start(ind[:], ind_in[:])
            offs = cp.tile([P, nch], i32)
            nc.sync.dma_start(offs[:], offs_in[:])
            offsA = cp.tile([P, ldch], i32)
            nc.sync.dma_start(offsA[:], offsA_in[:])
            offsB = cp.tile([P, ldch], i32)
            nc.sync.dma_start(offsB[:], offsB_in[:])

            aggT = cp.tile([P, sh], f32)       # layer activations, [f, dst] transposed
            logits_sb = cp.tile([P, ldch], f32)

            def emit_shard_block(psum_h, b, rb, layer):
                """psum_h [f, rows] -> (transpose, dinv-scale for layers<3) ->
                shard[layer] rows, bf16 node-major."""
                t1 = wp.tile([P, P], bf16, tag="t1")
                nc.scalar.activation(t1[:, :rb], psum_h[:, :rb],
                                     mybir.ActivationFunctionType.Copy)
                pt = ptr.tile([P, P], bf16, tag="pt")
                nc.tensor.transpose(pt[:rb, :], t1[:, :rb], ident[:])
                nc.vector.tensor_scalar_mul(shard_sb[:rb, b, :], pt[:rb, :],
                                            dv[:rb, b:b + 1])
                nc.sync.dma_start(shard[layer][b * P:b * P + rb, :],
                                  shard_sb[:rb, b, :])

            # ---- layer 0 table: htil0 = dinv * (x @ W0) ----
            with tc.tile_pool(name="xp", bufs=1) as xp:
                xt = xp.tile([P, KIN, sh], f32)
                for k in range(KIN):
                    nc.sync.dma_start(xt[:, k, :], xT[k, :, :])
                for b in range(nb):
                    rb = min(P, sh - b * P)
                    ph = pwm.tile([P, P], f32, tag="ph")
                    for k in range(KIN):
                        nc.tensor.matmul(ph[:, :rb], w0[:, k, :],
                                         xt[:, k, b * P:b * P + rb],
                                         start=(k == 0), stop=(k == KIN - 1))
                    emit_shard_block(ph, b, rb, 0)

            def do_allgather(layer):
                nc.gpsimd.collective_compute(
                    "AllGather", mybir.AluOpType.bypass, replica_groups=rg,
                    ins=[shard[layer].ap().opt()], outs=[full[layer].ap().opt()])

            def do_aggregation(layer, relu):
                """full[layer] -> aggT (with bias+relu) ; layer 2 writes z via
                emit path instead."""
                ch0 = 0
                for b in range(nb):
                    rb = min(P, sh - b * P)
                    pg = pagg.tile([P, P], f32, tag="pg")
                    k = int(kb[b])
                    nc.tensor.matmul(pg[:], shard_sb[:, b, :],
                                     diag[:, b * P:(b + 1) * P],
                                     start=True, stop=False)
                    for j in range(k):
                        c = ch0 + j
                        m = mp.tile([P, P], bf16, tag="m")
                        gi = nc.gpsimd.indirect_dma_start(
                            out=m[:], out_offset=None,
                            in_=full[layer][:, :],
                            in_offset=bass.IndirectOffsetOnAxis(
                                ap=offs[:, c:c + 1], axis=0))
                        gi.ins.single_packet = SINGLE_PACKET
                        nc.tensor.matmul(pg[:], m[:], ind[:, c * P:(c + 1) * P],
                                         start=False, stop=(j == k - 1))
                    ch0 += k
                    if relu:
                        nc.scalar.activation(
                            aggT[:, b * P:b * P + rb], pg[:, :rb],
                            mybir.ActivationFunctionType.Relu,
                            bias=bc[:, layer:layer + 1])
                    else:
                        # z block: add bias, then transpose out to z table
                        zt = wp.tile([P, P], bf16, tag="zt")
                        nc.vector.tensor_scalar_add(zt[:, :rb], pg[:, :rb],
                                                    bc[:, layer:layer + 1])
                        pt = ptr.tile([P, P], bf16, tag="pt")
                        nc.tensor.transpose(pt[:rb, :], zt[:, :rb], ident[:])
                        t2 = wp.tile([P, P], bf16, tag="zt2")
                        nc.vector.tensor_copy(t2[:rb, :], pt[:rb, :])
                        nc.sync.dma_start(shard[3][b * P:b * P + rb, :], t2[:rb, :])

            def do_weight_matmul(w, layer):
                for b in range(nb):
                    rb = min(P, sh - b * P)
                    ph = pwm.tile([P, P], f32, tag="ph")
                    nc.tensor.matmul(ph[:, :rb], w[:], aggT[:, b * P:b * P + rb],
                                     start=True, stop=True)
                    emit_shard_block(ph, b, rb, layer)

            stage = int(os.environ.get("GCN_STAGE", "5"))
            nc.gpsimd.memset(logits_sb[:], 0.0)
            if stage >= 1:
                do_allgather(0)
            if stage >= 2:
                do_aggregation(0, relu=True)
            if stage >= 3:
                do_weight_matmul(w1, 1)
                do_allgather(1)
                do_aggregation(1, relu=True)
            if stage >= 4:
                do_weight_matmul(w2, 2)
                do_allgather(2)
                do_aggregation(2, relu=False)   # writes z shard (layer tag 3)
                do_allgather(3)

            if stage >= 5:
                # ---- decode ----
                scratch = cp.tile([P, P], f32)
                for c in range(ldch):
                    za = mp.tile([P, P], bf16, tag="za")
                    ga = nc.gpsimd.indirect_dma_start(
                        out=za[:], out_offset=None, in_=full[3][:, :],
                        in_offset=bass.IndirectOffsetOnAxis(ap=offsA[:, c:c + 1], axis=0))
                    ga.ins.single_packet = SINGLE_PACKET
                    zb = mp.tile([P, P], bf16, tag="zb")
                    gb = nc.gpsimd.indirect_dma_start(
                        out=zb[:], out_offset=None, in_=full[3][:, :],
                        in_offset=bass.IndirectOffsetOnAxis(ap=offsB[:, c:c + 1], axis=0))
                    gb.ins.single_packet = SINGLE_PACKET
                    nc.vector.tensor_tensor(
                        out=scratch[:], in0=za[:], in1=zb[:],
                        op=mybir.AluOpType.mult)
                    nc.vector.tensor_reduce(
                        out=logits_sb[:, c:c + 1], in_=scratch[:],
                        axis=mybir.AxisListType.X, op=mybir.AluOpType.add)
            elif stage >= 2:
                # debug: dump first aggT columns
                nc.vector.tensor_copy(logits_sb[:, :min(ldch, 4)],
                                      aggT[:, :min(ldch, 4)])
            nc.sync.dma_start(logits_out[:], logits_sb[:])

    nc.compile()
    return nc


def _run(x, edge_index, edge_label_index, W0, b0, W1, b1, W2, b2):
    n, f_in = x.shape
    sh = n // NC
    deg = np.bincount(edge_index[1].astype(np.int64), minlength=n).astype(np.float64) + 1.0
    dinv = (1.0 / np.sqrt(deg)).astype(np.float32)

    meta = _build_plan(n, edge_index, edge_label_index, dinv)
    nc = _build_bass(n, f_in, meta)

    eye = np.eye(P, dtype=ml_dtypes.bfloat16)
    bcol = np.stack([b0, b1, b2], axis=1).astype(np.float32)  # [128, 3]
    nb = meta["nb"]
    dvb = np.zeros((NC, P, nb), np.float32)
    for c in range(NC):
        d = dinv[c * sh:(c + 1) * sh]
        d = np.pad(d, (0, nb * P - sh))
        dvb[c] = d.reshape(nb, P).T
    KIN = f_in // P

    diags = np.zeros((NC, P, nb * P), np.float32)
    for c in range(NC):
        for b in range(nb):
            np.fill_diagonal(diags[c, :, b * P:(b + 1) * P], dvb[c, :, b])
    diags = diags.astype(ml_dtypes.bfloat16)

    in_maps = []
    for c in range(NC):
        xs = x[c * sh:(c + 1) * sh].astype(np.float32)        # [sh, f_in]
        xT = np.ascontiguousarray(xs.T.reshape(KIN, P, sh))
        in_maps.append({
            "xT": xT,
            "W0": np.ascontiguousarray(W0.reshape(KIN, P, P).astype(np.float32)),
            "W1": W1.astype(np.float32), "W2": W2.astype(np.float32),
            "bcols": bcol, "dinv_blk": dvb[c], "ident": eye,
            "ind": np.ascontiguousarray(meta["ind"][c]),
            "diag": np.ascontiguousarray(diags[c]),
            "offs": np.ascontiguousarray(meta["offs"][c]),
            "offsA": np.ascontiguousarray(meta["offsA"][c]),
            "offsB": np.ascontiguousarray(meta["offsB"][c]),
        })

    res = run_bass_kernel_spmd(nc, in_maps, core_ids=list(range(NC)),
                               trace=bool(os.environ.get("GCN_TRACE")))
    lsh, ldch = meta["lsh"], meta["ldch"]
    outs = []
    for c in range(NC):
        lg = res.results[c]["logits"]          # [P, ldch], label l=c*P+p at (p,ch)
        outs.append(lg.T.reshape(-1)[:lsh])
    logits = np.concatenate(outs).astype(np.float32)
    return logits, res


def kernel(x, edge_index, edge_label_index, W0, b0, W1, b1, W2, b2):
    logits, _ = _run(np.asarray(x), np.asarray(edge_index), np.asarray(edge_label_index),
                     np.asarray(W0), np.asarray(b0), np.asarray(W1), np.asarray(b1),
                     np.asarray(W2), np.asarray(b2))
    return logits

